# revision 16
# baseline (speedup 1.0000x reference)
"""Causal multi-head attention on 8 trn2 NeuronCores.

Problem: B=2, S=2048, D=1024, H=16 heads, HD=64. fp32 in/out.

Sharding: 8 cores = 2 (batch) x 4 (head groups of 4 heads).
Each core computes, for its batch b and head group g:
  Q^T,K^T  [256, 2048]  (d on partitions, seq on free)  = W^T-slice x
  V        [2048, 256+ones]  (natural, with a ones column per head)
  per 512-wide q chunk, per head:  S^T[k,q] = K^T.T @ Q^T  (PE, contraction 64),
  P~ = exp(S^T/8) (ACT), causal via block skipping + one merged gpsimd
  affine_select per diagonal 2-block group, PV: ctx_raw^T[d,q] accumulated
  over k tiles with V_aug stationary (m=65; row 64 = softmax denominator).
  Normalize: DVE copies pv psum -> sbuf (releases the psum bank early),
  DVE reciprocal_approx_fast on the den row, gpsimd partition_broadcast,
  DVE mul -> ctx^T.  Then per chunk O_partial = ctx^T.T @ Wo_rows.
Host: sums the 4 head-group partials per batch and adds bo + bv @ Wo
(exact: the bv bias contributes the constant row vector bv @ Wo_g).

Emission order pipelines chunks: proj(ci) -> Wo(ci-1) -> attention(ci),
so the normalize chain of chunk ci-1 and Wo wait-time overlap the next
chunk's projections.  All input DMAs are single batched descriptors
(1 per weight tensor, 1 per x chunk); output stores are batched per
chunk and issued from the scalar-engine HWDGE queue so the sync queue
(input loads) is never blocked behind stores at the loop boundary.

Default mode "f16": ships x/Wq/Wk/Wv/Wo as fp16 and runs ALL matmuls in
native fp16 (1 cycle/row at any N, FWL fast weight loads) accumulating in
fp32 PSUM; p_t/qt/kt/vaug/ctxT/out in fp16 sbuf.  Normalization
internals (den, reciprocal, broadcast) in fp32.
"""

import math
import sys

if "/opt/trn_rl_repo" not in sys.path:
    sys.path.insert(0, "/opt/trn_rl_repo")

import numpy as np

import concourse.bacc as bacc
import concourse.bass as bass
import concourse.mybir as mybir
import concourse.tile as tile
from concourse.bass_utils import run_bass_kernel_spmd

B, S, D, H = 2, 2048, 1024, 16
HD = D // H  # 64
N_CORES = 8
HEADS_PER_CORE = H // 4  # 4
DG = HEADS_PER_CORE * HD  # 256 head dims per core
P = 128
CHUNK = 512  # q chunk width
N_KT = S // P  # 16 k tiles
N_CH = S // CHUNK  # 4 q chunks
F32 = mybir.dt.float32
F16 = mybir.dt.float16

_CACHE = {}


def _mm(dt_name):
    return {"f16": F16,
            "f32r": mybir.dt.float32r, "f16in": mybir.dt.float32r,
            "f32": mybir.dt.float32,
            "bf16": mybir.dt.bfloat16}[dt_name]


def _in_dt(dt_name):
    """dtype for the x / Wq / Wk / Wv inputs (DMA-traffic dominant)."""
    return F16 if dt_name in ("f16in", "f16") else _mm(dt_name)


def build_kernel(mm_dt="f16", unroll=1, ablate=()):
    """Build + compile the per-core SPMD program. unroll>1 wraps the body
    in a hardware loop (for pure device timing measurements)."""
    mdt = _mm(mm_dt)
    idt = _in_dt(mm_dt)
    odt = F16 if mm_dt == "f16" else F32

    nc = bacc.Bacc("TRN2", target_bir_lowering=False, debug=False)
    xT_d = nc.dram_tensor("xT", [D, S], idt, kind="ExternalInput")
    wq_d = nc.dram_tensor("wq", [D, DG], idt, kind="ExternalInput")
    wk_d = nc.dram_tensor("wk", [D, DG], idt, kind="ExternalInput")
    wv_d = nc.dram_tensor("wv", [D, DG], idt, kind="ExternalInput")
    wo_d = nc.dram_tensor("wo", [DG, D], mdt, kind="ExternalInput")
    # packed per-partition biases: col 0,1 = bq m-tiles, col 2,3 = bk m-tiles
    bias_d = nc.dram_tensor("bias", [P, 4], F32, kind="ExternalInput")
    o_d = nc.dram_tensor("o", [S, D], odt, kind="ExternalOutput")

    NDT = D // P  # 8 contraction tiles over D
    NMT = DG // P  # 2 m-tiles over the core's head dims (= head pairs)

    import contextlib
    with tile.TileContext(nc) as tc, contextlib.ExitStack() as stk:
        with tc.tile_pool(name="warm", bufs=1) as warm_p:
            # tiny pre-loop exp so the ACT table set is loaded once in the
            # preamble, not on every loop iteration
            wt = warm_p.tile([1, 8], F32, tag="warm", name="warm")
            nc.vector.memset(wt[:], 0.0)
            nc.scalar.activation(wt[:], wt[:],
                                 mybir.ActivationFunctionType.Exp)

        kb = _KernelBody(tc, nc, stk, mdt, idt, odt, xT_d, wq_d, wk_d,
                         wv_d, wo_d, bias_d, o_d, NDT, NMT, ablate)
        if unroll > 1:
            # software-pipelined loads: prologue loads before the loop;
            # each body reloads its inputs at the end (as soon as the last
            # reader of each tile retires) so transfers overlap the tail
            # and the next iteration starts computing immediately.
            kb.emit_loads()

            def body(_iv=None):
                for bi in range(bodies_per_iter):
                    kb.emit_compute(defer_tail=(bi < bodies_per_iter - 1))
                    kb.emit_loads(tail=True)

            # amortize the For_i all-engine barrier over several bodies;
            # adjacent bodies inside one iteration pipeline through plain
            # tile semaphores (loads of body k+1 overlap body k's tail)
            bodies_per_iter = 6 if unroll % 6 == 0 else (
                4 if unroll % 4 == 0 else 1)
            with tc.For_i(0, unroll // bodies_per_iter, 1):
                body()
        else:
            kb.emit_loads()
            kb.emit_compute()

    nc.compile()
    return nc


class _KernelBody:
    """Holds pools + persistent tiles; emits loads / compute separately so
    the loop can software-pipeline input loads against the compute tail."""

    def __init__(self, tc, nc, stk, mdt, idt, odt, xT_d, wq_d, wk_d, wv_d,
                 wo_d, bias_d, o_d, NDT, NMT, ablate=()):
        self.tc, self.nc = tc, nc
        self.mdt, self.idt, self.odt = mdt, idt, odt
        self.xT_d, self.wq_d, self.wk_d, self.wv_d = xT_d, wq_d, wk_d, wv_d
        self.wo_d, self.bias_d, self.o_d = wo_d, bias_d, o_d
        self.NDT, self.NMT = NDT, NMT
        self.ablate = ablate

        const = stk.enter_context(tc.tile_pool(name="const", bufs=1))
        sbuf = stk.enter_context(tc.tile_pool(name="sbuf", bufs=1))
        self.ptile_p = stk.enter_context(tc.tile_pool(name="ptile", bufs=8))
        self.craw_p = stk.enter_context(tc.tile_pool(name="craw", bufs=3))
        self.rec_p = stk.enter_context(tc.tile_pool(name="rec", bufs=3))
        self.recb_p = stk.enter_context(tc.tile_pool(name="recb", bufs=3))
        self.pending_tail = None
        self.out_p = stk.enter_context(tc.tile_pool(name="outp", bufs=2))
        self.qkv_ps = stk.enter_context(
            tc.tile_pool(name="qkv_ps", bufs=2, space="PSUM"))
        self.stp_ps = stk.enter_context(
            tc.tile_pool(name="stp_ps", bufs=2, space="PSUM"))
        self.pv_ps = stk.enter_context(
            tc.tile_pool(name="pv_ps", bufs=2, space="PSUM"))

        self.xt = const.tile([P, NDT * S], idt, tag="xt", name="xt")
        self.ws = {name: const.tile([P, NDT * DG], idt, tag=name, name=name)
                   for name in ("wq", "wk", "wv")}
        self.wo = const.tile([P, NMT * D], mdt, tag="wo", name="wo")
        self.bias_t = const.tile([P, 4], F32, tag="bias", name="bias")
        self.biases = {("bq", 0): self.bias_t[:, 0:1],
                       ("bq", 1): self.bias_t[:, 1:2],
                       ("bk", 0): self.bias_t[:, 2:3],
                       ("bk", 1): self.bias_t[:, 3:4]}
        self.ones_r = const.tile([P, HEADS_PER_CORE], mdt, tag="ones_r",
                                 name="ones_r")
        nc.vector.memset(self.ones_r[:], 1.0)
        # static causal mask (1 where q_local >= k, else 0); every trimmed
        # diagonal block uses a width-w prefix slice of it
        self.mask = const.tile([P, CHUNK], mdt, tag="mask", name="mask")
        nc.vector.memset(self.mask[:], 1.0)
        nc.gpsimd.affine_select(
            out=self.mask[:], in_=self.mask[:],
            compare_op=mybir.AluOpType.is_ge,
            fill=0.0, base=0, pattern=[[1, CHUNK]],
            channel_multiplier=-1)
        self.vaug = [sbuf.tile([P, HEADS_PER_CORE * (HD + 1)], mdt,
                               tag=f"vaug{j}", name=f"vaug{j}")
                     for j in range(N_KT)]
        self.qt = [sbuf.tile([P, S], mdt, tag=f"wqT{m}", name=f"wqT{m}")
                   for m in range(NMT)]
        self.kt = [sbuf.tile([P, S], mdt, tag=f"wkT{m}", name=f"wkT{m}")
                   for m in range(NMT)]
        self.ctxT = [sbuf.tile([P, S], mdt, tag=f"ctxT{m}", name=f"ctxT{m}")
                     for m in range(NMT)]

    # ---- input loads (batched descriptors, sync queue = loads only) -----
    def dma_w(self, name):
        d = {"wq": self.wq_d, "wk": self.wk_d, "wv": self.wv_d}[name]
        dst = self.ws[name][:].rearrange("p (k c) -> p k c", k=self.NDT)
        src = d.ap().rearrange("(k p) c -> p k c", k=self.NDT)
        self.nc.sync.dma_start(dst, src)

    def dma_xt(self, ci):
        csl = slice(CHUNK * ci, CHUNK * (ci + 1))
        dst = self.xt[:].rearrange("p (k s) -> p k s", k=self.NDT)[:, :, csl]
        src = self.xT_d.ap().rearrange("(k p) s -> p k s",
                                       k=self.NDT)[:, :, csl]
        self.nc.sync.dma_start(dst, src)

    def emit_loads(self, tail=False):
        nc = self.nc
        if tail:
            # reload order = earliest-tile-free first (the issue queue is
            # in-order; a late-freeing tile would block the others)
            self.dma_xt(0)
            nc.sync.dma_start(self.bias_t[:], self.bias_d.ap()[:, :])
            self.dma_xt(1)
            self.dma_xt(2)
            self.dma_w("wq")
            self.dma_w("wk")
            self.dma_xt(3)
            self.dma_w("wv")
            nc.sync.dma_start(
                self.wo[:].rearrange("p (m d) -> p m d", m=self.NMT),
                self.wo_d.ap().rearrange("(m p) d -> p m d", m=self.NMT))
            return
        self.dma_w("wv")
        self.dma_xt(0)
        self.dma_w("wq")
        self.dma_w("wk")
        nc.sync.dma_start(self.bias_t[:], self.bias_d.ap()[:, :])
        for ci in range(1, N_CH):
            self.dma_xt(ci)
        nc.sync.dma_start(
            self.wo[:].rearrange("p (m d) -> p m d", m=self.NMT),
            self.wo_d.ap().rearrange("(m p) d -> p m d", m=self.NMT))

    # ---- compute --------------------------------------------------------
    def xts(self, k, cols):
        return self.xt[:, S * k + cols.start:S * k + cols.stop]

    def wss(self, name, k, c0=0, c1=DG):
        return self.ws[name][:, DG * k + c0:DG * k + c1]

    def v_proj(self, j):
        nc = self.nc
        ps = self.qkv_ps.tile([P, CHUNK], F32, tag="proj", name="proj")
        for k in range(self.NDT):
            nc.tensor.matmul(
                ps[:, 0:DG],
                self.xts(k, slice(P * j, P * (j + 1))),
                self.wss("wv", k),
                start=(k == 0), stop=(k == self.NDT - 1))
        dst = self.vaug[j][:].rearrange("p (h x) -> p h x",
                                        h=HEADS_PER_CORE)
        srcp = ps[:, 0:DG].rearrange("p (h x) -> p h x", h=HEADS_PER_CORE)
        nc.vector.tensor_copy(dst[:, :, 0:HD], srcp[:, :, :])
        nc.vector.tensor_copy(
            dst[:, :, HD:HD + 1],
            self.ones_r[:].rearrange("p (h x) -> p h x", x=1))

    def qk_proj(self, ci):
        nc = self.nc
        for name, lst in (("wq", self.qt), ("wk", self.kt)):
            bname = "bq" if name == "wq" else "bk"
            for m in range(self.NMT):
                ps = self.qkv_ps.tile([P, CHUNK], F32, tag="proj",
                                      name="proj")
                for k in range(self.NDT):
                    nc.tensor.matmul(
                        ps[:],
                        self.wss(name, k, P * m, P * (m + 1)),
                        self.xts(k, slice(CHUNK * ci, CHUNK * (ci + 1))),
                        start=(k == 0), stop=(k == self.NDT - 1))
                nc.vector.tensor_scalar_add(
                    lst[m][:, CHUNK * ci:CHUNK * (ci + 1)], ps[:],
                    self.biases[(bname, m)])

    def wo_proj(self, ci):
        nc = self.nc
        ot = self.out_p.tile([P, 4 * D], self.odt, tag="ot", name="ot")
        for qi in range(4):
            i = 4 * ci + qi
            for e in range(2):
                ps = self.qkv_ps.tile([P, CHUNK], F32, tag="proj",
                                      name="proj")
                for m in range(self.NMT):
                    nc.tensor.matmul(
                        ps[:],
                        self.ctxT[m][:, P * i:P * (i + 1)],
                        self.wo[:, D * m + CHUNK * e:
                                D * m + CHUNK * (e + 1)],
                        start=(m == 0), stop=(m == self.NMT - 1))
                nc.any.tensor_copy(
                    ot[:, D * qi + CHUNK * e:D * qi + CHUNK * (e + 1)],
                    ps[:])
        # batched store for the whole chunk, issued from the scalar (ACT)
        # HWDGE queue: keeps the sync queue free for loads
        dst = self.o_d.ap()[CHUNK * ci:CHUNK * (ci + 1), :].rearrange(
            "(q p) d -> p q d", q=4)
        nc.scalar.dma_start(dst,
                            ot[:].rearrange("p (q d) -> p q d", q=4))

    def attention(self, ci):
        nc = self.nc
        ablate = self.ablate
        jmax = 4 * ci + 3  # last valid k tile for this chunk
        qsl = slice(CHUNK * ci, CHUNK * (ci + 1))
        for pair in range(self.NMT):
            pv = [self.pv_ps.tile([HD + 1, CHUNK], F32, tag="pv",
                                  name="pv") for _ in range(2)]
            for j0 in range(0, jmax + 1, 2):
                js = [j for j in (j0, j0 + 1) if j <= jmax]
                nj = len(js)
                pt = {}
                # live-column trim: on a diagonal block at offset dd the
                # columns q < 128*dd are entirely masked, so S/exp/PV only
                # touch the suffix [128*dd, CHUNK)
                def lo(j):
                    return P * (j - 4 * ci) if j >= 4 * ci else 0

                for hh in range(2):  # head within pair
                    psl = slice(64 * hh, 64 * (hh + 1))
                    st = self.stp_ps.tile([P, 2 * CHUNK], F32, tag="stp",
                                          name="stp")
                    for gi, j in enumerate(js):
                        nc.tensor.matmul(
                            st[:, CHUNK * gi + lo(j):CHUNK * (gi + 1)],
                            self.kt[pair][psl, P * j:P * (j + 1)],
                            self.qt[pair][psl,
                                          qsl.start + lo(j):qsl.stop],
                            start=True, stop=True)
                    p_t = self.ptile_p.tile([P, 2 * CHUNK], self.mdt,
                                            tag="ptile", name="ptile")
                    l0 = lo(js[0])
                    if "exp" in ablate:
                        nc.vector.tensor_copy(
                            p_t[:, l0:CHUNK * nj], st[:, l0:CHUNK * nj])
                    else:
                        nc.scalar.activation(
                            p_t[:, l0:CHUNK * nj], st[:, l0:CHUNK * nj],
                            mybir.ActivationFunctionType.Exp,
                            scale=0.125)
                    if j0 >= 4 * ci and "mask" not in ablate:
                        # per-block mask multiply on the trimmed suffix:
                        # with the dead prefix gone the predicate is
                        # uniformly q_local >= k, so a prefix slice of the
                        # static mask works for every diagonal block.
                        # DVE (2x fp16) keeps this off the gpsimd queue.
                        for gi, j in enumerate(js):
                            w = CHUNK - lo(j)
                            sel = p_t[:, CHUNK * (gi + 1) - w:
                                      CHUNK * (gi + 1)]
                            nc.vector.tensor_mul(sel, sel,
                                                 self.mask[:, 0:w])
                    pt[hh] = p_t
                for gi, j in enumerate(js):
                    for hh in range(2):
                        h = 2 * pair + hh
                        nc.tensor.matmul(
                            pv[hh][:, lo(j):],
                            self.vaug[j][:,
                                         (HD + 1) * h:(HD + 1) * (h + 1)],
                            pt[hh][:, CHUNK * gi + lo(j):CHUNK * (gi + 1)],
                            start=(j == 0), stop=(j == jmax))
            # ---- normalize phase 1: copy psum out (releases the pv bank
            # early).  Phase 2 (recip/broadcast/mul) is deferred past the
            # next chunk's projections so it never blocks the qk
            # evacuations in DVE program order.
            if "div" in ablate:
                for hh in range(2):
                    nc.vector.tensor_copy(
                        self.ctxT[pair][64 * hh:64 * (hh + 1), qsl],
                        pv[hh][0:HD, :])
            else:
                for hh in range(2):
                    craw = self.craw_p.tile([HD, CHUNK], F32,
                                            tag=f"craw{hh}",
                                            name=f"craw{hh}")
                    nc.vector.tensor_copy(craw[:], pv[hh][0:HD, :])
                    den = self.rec_p.tile([1, CHUNK], F32, tag=f"den{hh}",
                                          name=f"den{hh}")
                    # partition-base-shifting copy (64 -> 0); custom DVE
                    # ops need partition-0-based operands
                    nc.vector.tensor_copy(den[0:1, :],
                                          pv[hh][HD:HD + 1, :])
                    self.norm2_work.append((pair, hh, qsl, craw, den))

    def norm2_flush(self):
        nc = self.nc
        while self.norm2_work:
            pair, hh, qsl, craw, den = self.norm2_work.pop(0)
            rec = self.rec_p.tile([1, CHUNK], F32, tag=f"rec{hh}",
                                  name=f"rec{hh}")
            nc.vector.reciprocal_approx_fast(rec[0:1, :], den[0:1, :])
            recb = self.recb_p.tile([HD, CHUNK], F32, tag=f"recb{hh}",
                                    name=f"recb{hh}")
            nc.gpsimd.partition_broadcast(recb[0:HD, :], rec[0:1, :])
            nc.vector.tensor_mul(
                self.ctxT[pair][64 * hh:64 * (hh + 1), qsl],
                craw[0:HD, :],
                recb[0:HD, :])

    def emit_compute(self, defer_tail=False):
        # pipelined emission: proj(ci) -> norm2+Wo(ci-1) -> attention(ci).
        # The last chunk's normalize+Wo can be deferred into the NEXT
        # body's stream (after its proj(0)) so the PE never stalls on the
        # final divide chain.
        self.norm2_work = []
        for ci in range(N_CH):
            for j in range(4 * ci, 4 * ci + 4):
                self.v_proj(j)
            self.qk_proj(ci)
            if ci == 0 and self.pending_tail is not None:
                self.pending_tail()
                self.pending_tail = None
            if ci > 0:
                self.norm2_flush()
                self.wo_proj(ci - 1)
            self.attention(ci)
        if defer_tail:
            work = self.norm2_work
            self.norm2_work = []

            def tail():
                self.norm2_work, saved = work, self.norm2_work
                self.norm2_flush()
                self.norm2_work = saved
                self.wo_proj(N_CH - 1)

            self.pending_tail = tail
        else:
            self.norm2_flush()
            self.wo_proj(N_CH - 1)


def _shard_inputs(x, Wq, bq, Wk, bk, Wv, bv, Wo, bo):
    mm_dt = _CACHE.get("mm_dt", "f16")
    ndt = np.float16 if mm_dt in ("f16in", "f16") else np.float32
    wdt = np.float16 if mm_dt == "f16" else np.float32
    x = np.asarray(x, np.float32)
    in_maps = []
    for core in range(N_CORES):
        b, g = divmod(core, 4)
        ds = slice(DG * g, DG * (g + 1))
        bqc = np.asarray(bq, np.float32)[ds].reshape(2, P).T
        bkc = np.asarray(bk, np.float32)[ds].reshape(2, P).T
        in_maps.append({
            "xT": np.ascontiguousarray(x[b].T).astype(ndt),
            "wq": np.ascontiguousarray(
                np.asarray(Wq, np.float32)[:, ds]).astype(ndt),
            "wk": np.ascontiguousarray(
                np.asarray(Wk, np.float32)[:, ds]).astype(ndt),
            "wv": np.ascontiguousarray(
                np.asarray(Wv, np.float32)[:, ds]).astype(ndt),
            "wo": np.ascontiguousarray(
                np.asarray(Wo, np.float32)[ds, :]).astype(wdt),
            "bias": np.ascontiguousarray(
                np.concatenate([bqc, bkc], axis=1)),
        })
    return in_maps


def kernel(x, Wq, bq, Wk, bk, Wv, bv, Wo, bo):
    mm_dt = _CACHE.get("mm_dt", "f16")
    _CACHE["mm_dt"] = mm_dt
    if "nc" not in _CACHE:
        _CACHE["nc"] = build_kernel(mm_dt)
    nc = _CACHE["nc"]
    in_maps = _shard_inputs(x, Wq, bq, Wk, bk, Wv, bv, Wo, bo)
    res = run_bass_kernel_spmd(
        nc, in_maps, core_ids=list(range(N_CORES)), trace=False)
    out = np.zeros((B, S, D), np.float32)
    for core in range(N_CORES):
        out[core // 4] += res.results[core]["o"]
    # exact bias folding: +bo, + bv @ Wo (constant row vector)
    out += (np.asarray(bo, np.float32)
            + np.asarray(bv, np.float32) @ np.asarray(Wo, np.float32))
    return out


# revision 17
# speedup vs baseline: 1.0229x; 1.0229x over previous
"""Causal multi-head attention on 8 trn2 NeuronCores.

Problem: B=2, S=2048, D=1024, H=16 heads, HD=64. fp32 in/out.

Sharding: 8 cores = 2 (batch) x 4 (head groups of 4 heads).
Each core computes, for its batch b and head group g:
  Q^T,K^T  [256, 2048]  (d on partitions, seq on free)  = W^T-slice x
  V        [2048, 256+ones]  (natural, with a ones column per head)
  per 512-wide q chunk, per head:  S^T[k,q] = K^T.T @ Q^T  (PE, contraction 64),
  P~ = exp(S^T/8) (ACT), causal via block skipping + one merged gpsimd
  affine_select per diagonal 2-block group, PV: ctx_raw^T[d,q] accumulated
  over k tiles with V_aug stationary (m=65; row 64 = softmax denominator).
  Normalize: DVE copies pv psum -> sbuf (releases the psum bank early),
  DVE reciprocal_approx_fast on the den row, gpsimd partition_broadcast,
  DVE mul -> ctx^T.  Then per chunk O_partial = ctx^T.T @ Wo_rows.
Host: sums the 4 head-group partials per batch and adds bo + bv @ Wo
(exact: the bv bias contributes the constant row vector bv @ Wo_g).

Emission order pipelines chunks: proj(ci) -> Wo(ci-1) -> attention(ci),
so the normalize chain of chunk ci-1 and Wo wait-time overlap the next
chunk's projections.  All input DMAs are single batched descriptors
(1 per weight tensor, 1 per x chunk); output stores are batched per
chunk and issued from the scalar-engine HWDGE queue so the sync queue
(input loads) is never blocked behind stores at the loop boundary.

Default mode "f16": ships x/Wq/Wk/Wv/Wo as fp16 and runs ALL matmuls in
native fp16 (1 cycle/row at any N, FWL fast weight loads) accumulating in
fp32 PSUM; p_t/qt/kt/vaug/ctxT/out in fp16 sbuf.  Normalization
internals (den, reciprocal, broadcast) in fp32.
"""

import math
import sys

if "/opt/trn_rl_repo" not in sys.path:
    sys.path.insert(0, "/opt/trn_rl_repo")

import numpy as np

import concourse.bacc as bacc
import concourse.bass as bass
import concourse.mybir as mybir
import concourse.tile as tile
from concourse.bass_utils import run_bass_kernel_spmd

B, S, D, H = 2, 2048, 1024, 16
HD = D // H  # 64
N_CORES = 8
HEADS_PER_CORE = H // 4  # 4
DG = HEADS_PER_CORE * HD  # 256 head dims per core
P = 128
CHUNK = 512  # q chunk width
N_KT = S // P  # 16 k tiles
N_CH = S // CHUNK  # 4 q chunks
F32 = mybir.dt.float32
F16 = mybir.dt.float16

_CACHE = {}


def _mm(dt_name):
    return {"f16": F16,
            "f32r": mybir.dt.float32r, "f16in": mybir.dt.float32r,
            "f32": mybir.dt.float32,
            "bf16": mybir.dt.bfloat16}[dt_name]


def _in_dt(dt_name):
    """dtype for the x / Wq / Wk / Wv inputs (DMA-traffic dominant)."""
    return F16 if dt_name in ("f16in", "f16") else _mm(dt_name)


def build_kernel(mm_dt="f16", unroll=1, ablate=()):
    """Build + compile the per-core SPMD program. unroll>1 wraps the body
    in a hardware loop (for pure device timing measurements)."""
    mdt = _mm(mm_dt)
    idt = _in_dt(mm_dt)
    odt = F16 if mm_dt == "f16" else F32

    nc = bacc.Bacc("TRN2", target_bir_lowering=False, debug=False)
    xT_d = nc.dram_tensor("xT", [D, S], idt, kind="ExternalInput")
    wq_d = nc.dram_tensor("wq", [D, DG], idt, kind="ExternalInput")
    wk_d = nc.dram_tensor("wk", [D, DG], idt, kind="ExternalInput")
    wv_d = nc.dram_tensor("wv", [D, DG], idt, kind="ExternalInput")
    wo_d = nc.dram_tensor("wo", [DG, D], mdt, kind="ExternalInput")
    # packed per-partition biases: col 0,1 = bq m-tiles, col 2,3 = bk m-tiles
    bias_d = nc.dram_tensor("bias", [P, 4], F32, kind="ExternalInput")
    o_d = nc.dram_tensor("o", [S, D], odt, kind="ExternalOutput")

    NDT = D // P  # 8 contraction tiles over D
    NMT = DG // P  # 2 m-tiles over the core's head dims (= head pairs)

    import contextlib
    with tile.TileContext(nc) as tc, contextlib.ExitStack() as stk:
        with tc.tile_pool(name="warm", bufs=1) as warm_p:
            # tiny pre-loop exp so the ACT table set is loaded once in the
            # preamble, not on every loop iteration
            wt = warm_p.tile([1, 8], F32, tag="warm", name="warm")
            nc.vector.memset(wt[:], 0.0)
            nc.scalar.activation(wt[:], wt[:],
                                 mybir.ActivationFunctionType.Exp)

        kb = _KernelBody(tc, nc, stk, mdt, idt, odt, xT_d, wq_d, wk_d,
                         wv_d, wo_d, bias_d, o_d, NDT, NMT, ablate)
        if unroll > 1:
            # software-pipelined loads: prologue loads before the loop;
            # each body reloads its inputs at the end (as soon as the last
            # reader of each tile retires) so transfers overlap the tail
            # and the next iteration starts computing immediately.
            kb.emit_loads()

            def body(_iv=None):
                for bi in range(bodies_per_iter):
                    kb.emit_compute(defer_tail=(bi < bodies_per_iter - 1))
                    kb.emit_loads(tail=True)

            # amortize the For_i all-engine barrier over several bodies;
            # adjacent bodies inside one iteration pipeline through plain
            # tile semaphores (loads of body k+1 overlap body k's tail)
            bodies_per_iter = 6 if unroll % 6 == 0 else (
                4 if unroll % 4 == 0 else 1)
            with tc.For_i(0, unroll // bodies_per_iter, 1):
                body()
        else:
            kb.emit_loads()
            kb.emit_compute()

    nc.compile()
    return nc


class _KernelBody:
    """Holds pools + persistent tiles; emits loads / compute separately so
    the loop can software-pipeline input loads against the compute tail."""

    def __init__(self, tc, nc, stk, mdt, idt, odt, xT_d, wq_d, wk_d, wv_d,
                 wo_d, bias_d, o_d, NDT, NMT, ablate=()):
        self.tc, self.nc = tc, nc
        self.mdt, self.idt, self.odt = mdt, idt, odt
        self.xT_d, self.wq_d, self.wk_d, self.wv_d = xT_d, wq_d, wk_d, wv_d
        self.wo_d, self.bias_d, self.o_d = wo_d, bias_d, o_d
        self.NDT, self.NMT = NDT, NMT
        self.ablate = ablate

        const = stk.enter_context(tc.tile_pool(name="const", bufs=1))
        sbuf = stk.enter_context(tc.tile_pool(name="sbuf", bufs=1))
        self.ptile_p = stk.enter_context(tc.tile_pool(name="ptile", bufs=8))
        self.craw_p = stk.enter_context(tc.tile_pool(name="craw", bufs=3))
        self.rec_p = stk.enter_context(tc.tile_pool(name="rec", bufs=3))
        self.recb_p = stk.enter_context(tc.tile_pool(name="recb", bufs=3))
        self.pending_tail = None
        self.out_p = stk.enter_context(tc.tile_pool(name="outp", bufs=2))
        self.qkv_ps = stk.enter_context(
            tc.tile_pool(name="qkv_ps", bufs=2, space="PSUM"))
        self.stp_ps = stk.enter_context(
            tc.tile_pool(name="stp_ps", bufs=2, space="PSUM"))
        self.pv_ps = stk.enter_context(
            tc.tile_pool(name="pv_ps", bufs=2, space="PSUM"))

        self.xt = const.tile([P, NDT * S], idt, tag="xt", name="xt")
        self.ws = {name: const.tile([P, NDT * DG], idt, tag=name, name=name)
                   for name in ("wq", "wk", "wv")}
        self.wo = const.tile([P, NMT * D], mdt, tag="wo", name="wo")
        self.bias_t = const.tile([P, 4], F32, tag="bias", name="bias")
        self.biases = {("bq", 0): self.bias_t[:, 0:1],
                       ("bq", 1): self.bias_t[:, 1:2],
                       ("bk", 0): self.bias_t[:, 2:3],
                       ("bk", 1): self.bias_t[:, 3:4]}
        self.ones_r = const.tile([P, HEADS_PER_CORE], mdt, tag="ones_r",
                                 name="ones_r")
        nc.vector.memset(self.ones_r[:], 1.0)
        # static causal mask (1 where q_local >= k, else 0); every trimmed
        # diagonal block uses a width-w prefix slice of it
        self.mask = const.tile([P, CHUNK], mdt, tag="mask", name="mask")
        nc.vector.memset(self.mask[:], 1.0)
        nc.gpsimd.affine_select(
            out=self.mask[:], in_=self.mask[:],
            compare_op=mybir.AluOpType.is_ge,
            fill=0.0, base=0, pattern=[[1, CHUNK]],
            channel_multiplier=-1)
        self.vaug = [sbuf.tile([P, HEADS_PER_CORE * (HD + 1)], mdt,
                               tag=f"vaug{j}", name=f"vaug{j}")
                     for j in range(N_KT)]
        self.qt = [sbuf.tile([P, S], mdt, tag=f"wqT{m}", name=f"wqT{m}")
                   for m in range(NMT)]
        self.kt = [sbuf.tile([P, S], mdt, tag=f"wkT{m}", name=f"wkT{m}")
                   for m in range(NMT)]
        self.ctxT = [sbuf.tile([P, S], mdt, tag=f"ctxT{m}", name=f"ctxT{m}")
                     for m in range(NMT)]

    # ---- input loads (batched descriptors, sync queue = loads only) -----
    def dma_w(self, name):
        d = {"wq": self.wq_d, "wk": self.wk_d, "wv": self.wv_d}[name]
        dst = self.ws[name][:].rearrange("p (k c) -> p k c", k=self.NDT)
        src = d.ap().rearrange("(k p) c -> p k c", k=self.NDT)
        self.nc.sync.dma_start(dst, src)

    def dma_xt(self, ci):
        csl = slice(CHUNK * ci, CHUNK * (ci + 1))
        dst = self.xt[:].rearrange("p (k s) -> p k s", k=self.NDT)[:, :, csl]
        src = self.xT_d.ap().rearrange("(k p) s -> p k s",
                                       k=self.NDT)[:, :, csl]
        self.nc.sync.dma_start(dst, src)

    def emit_loads(self, tail=False):
        nc = self.nc
        if tail:
            # reload order = earliest-tile-free first (the issue queue is
            # in-order; a late-freeing tile would block the others)
            self.dma_xt(0)
            nc.sync.dma_start(self.bias_t[:], self.bias_d.ap()[:, :])
            self.dma_xt(1)
            self.dma_xt(2)
            self.dma_w("wq")
            self.dma_w("wk")
            self.dma_xt(3)
            self.dma_w("wv")
            nc.sync.dma_start(
                self.wo[:].rearrange("p (m d) -> p m d", m=self.NMT),
                self.wo_d.ap().rearrange("(m p) d -> p m d", m=self.NMT))
            return
        self.dma_w("wv")
        self.dma_xt(0)
        self.dma_w("wq")
        self.dma_w("wk")
        nc.sync.dma_start(self.bias_t[:], self.bias_d.ap()[:, :])
        for ci in range(1, N_CH):
            self.dma_xt(ci)
        nc.sync.dma_start(
            self.wo[:].rearrange("p (m d) -> p m d", m=self.NMT),
            self.wo_d.ap().rearrange("(m p) d -> p m d", m=self.NMT))

    # ---- compute --------------------------------------------------------
    def xts(self, k, cols):
        return self.xt[:, S * k + cols.start:S * k + cols.stop]

    def wss(self, name, k, c0=0, c1=DG):
        return self.ws[name][:, DG * k + c0:DG * k + c1]

    def v_proj(self, j):
        nc = self.nc
        ps = self.qkv_ps.tile([P, CHUNK], F32, tag="proj", name="proj")
        for k in range(self.NDT):
            nc.tensor.matmul(
                ps[:, 0:DG],
                self.xts(k, slice(P * j, P * (j + 1))),
                self.wss("wv", k),
                start=(k == 0), stop=(k == self.NDT - 1))
        dst = self.vaug[j][:].rearrange("p (h x) -> p h x",
                                        h=HEADS_PER_CORE)
        srcp = ps[:, 0:DG].rearrange("p (h x) -> p h x", h=HEADS_PER_CORE)
        nc.vector.tensor_copy(dst[:, :, 0:HD], srcp[:, :, :])
        nc.vector.tensor_copy(
            dst[:, :, HD:HD + 1],
            self.ones_r[:].rearrange("p (h x) -> p h x", x=1))

    def qk_proj(self, ci):
        nc = self.nc
        for name, lst in (("wq", self.qt), ("wk", self.kt)):
            bname = "bq" if name == "wq" else "bk"
            for m in range(self.NMT):
                ps = self.qkv_ps.tile([P, CHUNK], F32, tag="proj",
                                      name="proj")
                for k in range(self.NDT):
                    nc.tensor.matmul(
                        ps[:],
                        self.wss(name, k, P * m, P * (m + 1)),
                        self.xts(k, slice(CHUNK * ci, CHUNK * (ci + 1))),
                        start=(k == 0), stop=(k == self.NDT - 1))
                nc.vector.tensor_scalar_add(
                    lst[m][:, CHUNK * ci:CHUNK * (ci + 1)], ps[:],
                    self.biases[(bname, m)])

    def wo_proj(self, ci):
        nc = self.nc
        ot = self.out_p.tile([P, 4 * D], self.odt, tag="ot", name="ot")
        for qi in range(4):
            i = 4 * ci + qi
            for e in range(2):
                ps = self.qkv_ps.tile([P, CHUNK], F32, tag="proj",
                                      name="proj")
                for m in range(self.NMT):
                    nc.tensor.matmul(
                        ps[:],
                        self.ctxT[m][:, P * i:P * (i + 1)],
                        self.wo[:, D * m + CHUNK * e:
                                D * m + CHUNK * (e + 1)],
                        start=(m == 0), stop=(m == self.NMT - 1))
                nc.any.tensor_copy(
                    ot[:, D * qi + CHUNK * e:D * qi + CHUNK * (e + 1)],
                    ps[:])
        # batched store for the whole chunk, issued from the scalar (ACT)
        # HWDGE queue: keeps the sync queue free for loads
        dst = self.o_d.ap()[CHUNK * ci:CHUNK * (ci + 1), :].rearrange(
            "(q p) d -> p q d", q=4)
        nc.scalar.dma_start(dst,
                            ot[:].rearrange("p (q d) -> p q d", q=4))

    def attention(self, ci):
        nc = self.nc
        ablate = self.ablate
        jmax = 4 * ci + 3  # last valid k tile for this chunk
        qsl = slice(CHUNK * ci, CHUNK * (ci + 1))
        for pair in range(self.NMT):
            pv = [self.pv_ps.tile([HD + 1, CHUNK], F32, tag="pv",
                                  name="pv") for _ in range(2)]
            for j0 in range(0, jmax + 1, 2):
                js = [j for j in (j0, j0 + 1) if j <= jmax]
                nj = len(js)
                pt = {}
                # live-column trim: on a diagonal block at offset dd the
                # columns q < 128*dd are entirely masked, so S/exp/PV only
                # touch the suffix [128*dd, CHUNK)
                def lo(j):
                    return P * (j - 4 * ci) if j >= 4 * ci else 0

                for hh in range(2):  # head within pair
                    psl = slice(64 * hh, 64 * (hh + 1))
                    st = self.stp_ps.tile([P, 2 * CHUNK], F32, tag="stp",
                                          name="stp")
                    for gi, j in enumerate(js):
                        nc.tensor.matmul(
                            st[:, CHUNK * gi + lo(j):CHUNK * (gi + 1)],
                            self.kt[pair][psl, P * j:P * (j + 1)],
                            self.qt[pair][psl,
                                          qsl.start + lo(j):qsl.stop],
                            start=True, stop=True)
                    p_t = self.ptile_p.tile([P, 2 * CHUNK], self.mdt,
                                            tag="ptile", name="ptile")
                    l0 = lo(js[0])
                    if "exp" in ablate:
                        nc.vector.tensor_copy(
                            p_t[:, l0:CHUNK * nj], st[:, l0:CHUNK * nj])
                    else:
                        nc.scalar.activation(
                            p_t[:, l0:CHUNK * nj], st[:, l0:CHUNK * nj],
                            mybir.ActivationFunctionType.Exp,
                            scale=0.125)
                    if j0 >= 4 * ci and "mask" not in ablate:
                        # per-block select on the trimmed suffix: with the
                        # dead prefix gone the predicate is uniformly
                        # q_local >= k for every diagonal block
                        for gi, j in enumerate(js):
                            w = CHUNK - lo(j)
                            sel = p_t[:, CHUNK * (gi + 1) - w:
                                      CHUNK * (gi + 1)]
                            nc.gpsimd.affine_select(
                                out=sel,
                                in_=sel,
                                compare_op=mybir.AluOpType.is_ge,
                                fill=0.0, base=0,
                                pattern=[[1, w]],
                                channel_multiplier=-1)
                    pt[hh] = p_t
                for gi, j in enumerate(js):
                    for hh in range(2):
                        h = 2 * pair + hh
                        nc.tensor.matmul(
                            pv[hh][:, lo(j):],
                            self.vaug[j][:,
                                         (HD + 1) * h:(HD + 1) * (h + 1)],
                            pt[hh][:, CHUNK * gi + lo(j):CHUNK * (gi + 1)],
                            start=(j == 0), stop=(j == jmax))
            # ---- normalize phase 1: copy psum out (releases the pv bank
            # early).  Phase 2 (recip/broadcast/mul) is deferred past the
            # next chunk's projections so it never blocks the qk
            # evacuations in DVE program order.
            if "div" in ablate:
                for hh in range(2):
                    nc.vector.tensor_copy(
                        self.ctxT[pair][64 * hh:64 * (hh + 1), qsl],
                        pv[hh][0:HD, :])
            else:
                for hh in range(2):
                    craw = self.craw_p.tile([HD, CHUNK], F32,
                                            tag=f"craw{hh}",
                                            name=f"craw{hh}")
                    nc.vector.tensor_copy(craw[:], pv[hh][0:HD, :])
                    den = self.rec_p.tile([1, CHUNK], F32, tag=f"den{hh}",
                                          name=f"den{hh}")
                    # partition-base-shifting copy (64 -> 0); custom DVE
                    # ops need partition-0-based operands
                    nc.vector.tensor_copy(den[0:1, :],
                                          pv[hh][HD:HD + 1, :])
                    self.norm2_work.append((pair, hh, qsl, craw, den))

    def norm2_flush(self):
        nc = self.nc
        while self.norm2_work:
            pair, hh, qsl, craw, den = self.norm2_work.pop(0)
            rec = self.rec_p.tile([1, CHUNK], F32, tag=f"rec{hh}",
                                  name=f"rec{hh}")
            nc.vector.reciprocal_approx_fast(rec[0:1, :], den[0:1, :])
            recb = self.recb_p.tile([HD, CHUNK], F32, tag=f"recb{hh}",
                                    name=f"recb{hh}")
            nc.gpsimd.partition_broadcast(recb[0:HD, :], rec[0:1, :])
            nc.vector.tensor_mul(
                self.ctxT[pair][64 * hh:64 * (hh + 1), qsl],
                craw[0:HD, :],
                recb[0:HD, :])

    def emit_compute(self, defer_tail=False):
        # pipelined emission: proj(ci) -> norm2+Wo(ci-1) -> attention(ci).
        # The last chunk's normalize+Wo can be deferred into the NEXT
        # body's stream (after its proj(0)) so the PE never stalls on the
        # final divide chain.
        self.norm2_work = []
        for ci in range(N_CH):
            for j in range(4 * ci, 4 * ci + 4):
                self.v_proj(j)
            self.qk_proj(ci)
            if ci == 0 and self.pending_tail is not None:
                self.pending_tail()
                self.pending_tail = None
            if ci > 0:
                self.norm2_flush()
                self.wo_proj(ci - 1)
            self.attention(ci)
        if defer_tail:
            work = self.norm2_work
            self.norm2_work = []

            def tail():
                self.norm2_work, saved = work, self.norm2_work
                self.norm2_flush()
                self.norm2_work = saved
                self.wo_proj(N_CH - 1)

            self.pending_tail = tail
        else:
            self.norm2_flush()
            self.wo_proj(N_CH - 1)


def _shard_inputs(x, Wq, bq, Wk, bk, Wv, bv, Wo, bo):
    mm_dt = _CACHE.get("mm_dt", "f16")
    ndt = np.float16 if mm_dt in ("f16in", "f16") else np.float32
    wdt = np.float16 if mm_dt == "f16" else np.float32
    x = np.asarray(x, np.float32)
    in_maps = []
    for core in range(N_CORES):
        b, g = divmod(core, 4)
        ds = slice(DG * g, DG * (g + 1))
        bqc = np.asarray(bq, np.float32)[ds].reshape(2, P).T
        bkc = np.asarray(bk, np.float32)[ds].reshape(2, P).T
        in_maps.append({
            "xT": np.ascontiguousarray(x[b].T).astype(ndt),
            "wq": np.ascontiguousarray(
                np.asarray(Wq, np.float32)[:, ds]).astype(ndt),
            "wk": np.ascontiguousarray(
                np.asarray(Wk, np.float32)[:, ds]).astype(ndt),
            "wv": np.ascontiguousarray(
                np.asarray(Wv, np.float32)[:, ds]).astype(ndt),
            "wo": np.ascontiguousarray(
                np.asarray(Wo, np.float32)[ds, :]).astype(wdt),
            "bias": np.ascontiguousarray(
                np.concatenate([bqc, bkc], axis=1)),
        })
    return in_maps


def kernel(x, Wq, bq, Wk, bk, Wv, bv, Wo, bo):
    mm_dt = _CACHE.get("mm_dt", "f16")
    _CACHE["mm_dt"] = mm_dt
    if "nc" not in _CACHE:
        _CACHE["nc"] = build_kernel(mm_dt)
    nc = _CACHE["nc"]
    in_maps = _shard_inputs(x, Wq, bq, Wk, bk, Wv, bv, Wo, bo)
    res = run_bass_kernel_spmd(
        nc, in_maps, core_ids=list(range(N_CORES)), trace=False)
    out = np.zeros((B, S, D), np.float32)
    for core in range(N_CORES):
        out[core // 4] += res.results[core]["o"]
    # exact bias folding: +bo, + bv @ Wo (constant row vector)
    out += (np.asarray(bo, np.float32)
            + np.asarray(bv, np.float32) @ np.asarray(Wo, np.float32))
    return out
